# revision 12
# baseline (speedup 1.0000x reference)
"""Multi-head attention (B=2, S=2048, E=1024, H=16, D=64) on 8 TRN2 cores.

Sharding: tensor-parallel over heads. Core c owns heads {2c, 2c+1}:
  - Q/K/V projections column-sharded (128 cols each per core)
  - attention for the core's 2 heads (both batches)
  - out-projection row-sharded (128 rows of Wo) -> partial [4096,1024] f16
  - host sums the 8 partials and adds bo.

On-chip layout (everything "transposed"):
  - host passes xT [1024, 4096] (E-major, fp16) so the contraction dim
    lands on SBUF partitions with no on-device transpose of x
  - projections produce Q^T, K^T [128, 4096] (head-dim on partitions);
    V^T is PE-transposed to token-major Vtm tiles [tok, mt, 130] laid
    out [V_h0(0:64) | ones(64) | V_h1(65:129) | ones(129)] so each
    head's AV matmul reads a contiguous 65-col [V|ones] stationary
    whose ones column accumulates the softmax denominator
  - scores are computed transposed: scores^T[key, q]; the key-padding
    mask folds into the exp() per-partition bias
  - AV emits av[0:64]=Y^T, av[64]=denominator; normalization divides on
    DVE (cross-partition-base write) into a single merged YT [128, M]
    so the out-projection runs full-K=128 matmuls (half the PE time of
    per-head K=64 pairs).

Engine discipline:
  - ACT (scalar) is the attention wall: 128 exp instrs of [128,1024]
    (~147us total). It must stay a pure-exp stream during passes; the
    only non-exp ACT work is phase-1 bias-adds (ACT idle then) and
    tail copies (after the last exp).
  - PE fills its exp-wait gaps with deferred batch-1 projections and
    ready out-projection tiles (paced one unit per ~3 key tiles).
  - psum_sc is allocated before phase 1's pool so pass-0 score matmuls
    get fresh banks with no dependency on phase-1 drain.
  - normalization: denom row -> DVE copy [1,512] -> DVE fast reciprocal
    -> GPSIMD partition_broadcast [64,512] -> DVE multiply into YT
    (no DRAM bounce).
"""

import os
import numpy as np

B, S, E, H, D = 2, 2048, 1024, 16, 64
M = B * S            # 4096 tokens
P = 128              # partitions
NCORES = 8
KC = E // P          # 8 contraction chunks for projections
MCH = 512            # token chunk for projections
QW = 1024            # query width per attention pass
QCH = 512            # query chunk (psum-bank sized)
NKT = S // P         # 16 key tiles per batch
VW = 2 * (D + 1)     # 130 Vtm cols: [V0 | ones | V1 | ones]
NEG = -1.0e30

LAST_RESULTS = None  # BassKernelResults of the most recent run (for test harness)
_PROGRAM = None


def _build_program():
    import concourse.bass as bass
    import concourse.tile as tile
    from concourse import bacc, mybir
    from concourse.masks import make_identity

    f32 = mybir.dt.float32
    f16 = mybir.dt.float16
    Exp = mybir.ActivationFunctionType.Exp

    nc = bacc.Bacc(
        "TRN2",
        target_bir_lowering=False,
        debug=False,
        enable_asserts=False,
        num_devices=NCORES,
    )

    xT_d = nc.dram_tensor("xT", (E, M), f16, kind="ExternalInput").ap()
    wq_d = nc.dram_tensor("wq", (P, KC, P), f16, kind="ExternalInput").ap()
    wk_d = nc.dram_tensor("wk", (P, KC, P), f16, kind="ExternalInput").ap()
    wv_d = nc.dram_tensor("wv", (P, KC, P), f16, kind="ExternalInput").ap()
    wo_d = nc.dram_tensor("wo", (P, E), f16, kind="ExternalInput").ap()
    bq_d = nc.dram_tensor("bq", (P, 1), f32, kind="ExternalInput").ap()
    bk_d = nc.dram_tensor("bk", (P, 1), f32, kind="ExternalInput").ap()
    bv_d = nc.dram_tensor("bv", (P, 1), f32, kind="ExternalInput").ap()
    maskT_d = nc.dram_tensor("maskT", (P, B * 16), f32, kind="ExternalInput").ap()
    out_d = nc.dram_tensor("out", (M, E), f16, kind="ExternalOutput").ap()

    with tile.TileContext(nc) as tc:
        with (
            # psum_sc first: banks 0-3, untouched by phase 1, so pass-0
            # score matmuls never wait on phase-1 psum drain
            tc.tile_pool(name="psum_sc", bufs=2, space="PSUM") as psum_sc,
            tc.tile_pool(name="consts", bufs=1) as consts,
            tc.tile_pool(name="big", bufs=1) as big,
            tc.tile_pool(name="xt_pool", bufs=8) as xt_pool,
            tc.tile_pool(name="vt_pool", bufs=2) as vt_pool,
            tc.tile_pool(name="pt_pool", bufs=18) as pt_pool,
            tc.tile_pool(name="r_pool", bufs=2) as r_pool,
            tc.tile_pool(name="out_pool", bufs=6) as out_pool,
        ):
            # ---- constants ----
            wq_sb = consts.tile([P, KC, P], f16)
            wk_sb = consts.tile([P, KC, P], f16)
            wv_sb = consts.tile([P, KC, P], f16)
            wo_sb = consts.tile([P, E], f16)
            bq_sb = consts.tile([P, 1], f32)
            bk_sb = consts.tile([P, 1], f32)
            bv_sb = consts.tile([P, 1], f32)
            mask_sb = consts.tile([P, B * 16], f32)
            ident = consts.tile([P, P], f16)

            # first matmul is gated on wq + first xt tile: issue those
            # first (sync and scalar HWDGE queues in parallel)
            xt00 = xt_pool.tile([P, MCH], f16, tag="xt", name="xt")
            nc.sync.dma_start(xt00, xT_d[0:P, 0:MCH])
            nc.scalar.dma_start(wq_sb, wq_d)
            nc.sync.dma_start(wk_sb, wk_d)
            nc.scalar.dma_start(wv_sb, wv_d)
            nc.gpsimd.dma_start(wo_sb, wo_d)
            nc.gpsimd.dma_start(bq_sb, bq_d)
            nc.gpsimd.dma_start(bk_sb, bk_d)
            nc.gpsimd.dma_start(bv_sb, bv_d)
            nc.gpsimd.dma_start(mask_sb, maskT_d)
            make_identity(nc, ident)

            # ---- big persistent activations ----
            QT = big.tile([P, M], f16)       # Q^T: head-dims on partitions
            KT = big.tile([P, M], f16)
            Vtm = big.tile([P, M // P, VW], f16)   # token-major V tiles
            YT = big.tile([P, M], f16)       # merged normalized attn out^T

            nc.vector.memset(Vtm[:, :, D : D + 1], 1.0)
            nc.vector.memset(Vtm[:, :, VW - 1 : VW], 1.0)

            # ---------------------------------------------------------------
            # phase 1: batch-0 projections on psum banks 4-7 (q,k,v,vtp).
            # Single m-chunk groups; Q/K bias-adds ride the idle ACT engine;
            # group g's V transposes are deferred into group g+1's matmul
            # stream so the PE never waits on the V bias-add.
            # ---------------------------------------------------------------
            with tc.tile_pool(name="psum_p1", bufs=3, space="PSUM") as psum_p1:

                def emit_vtrans(pool, mc, vt):
                    # transpose vt [128, 512] into 4 token-major Vtm tiles;
                    # single 1-bank psum tile, two strided batch copies
                    mt0 = mc * (MCH // P)
                    vtp = pool.tile([P, MCH // P, P], f16, tag="vtp", bufs=1,
                                    name="vtp")
                    for j in range(MCH // P):
                        nc.tensor.transpose(
                            vtp[:, j, :], vt[:, bass.ts(j, P)], ident
                        )
                    nc.vector.tensor_copy(
                        Vtm[:, mt0 : mt0 + MCH // P, 0:D], vtp[:, :, 0:D]
                    )
                    nc.vector.tensor_copy(
                        Vtm[:, mt0 : mt0 + MCH // P, D + 1 : 2 * D + 1],
                        vtp[:, :, D : 2 * D],
                    )

                P1ENG = [nc.sync, nc.scalar]
                pending_vt = []
                for mc in range(S // MCH):
                    msl = bass.ts(mc, MCH)
                    qp = psum_p1.tile([P, MCH], f32, tag="qp", bufs=1, name="qp")
                    kp = psum_p1.tile([P, MCH], f32, tag="kp", bufs=1, name="kp")
                    vp = psum_p1.tile([P, MCH], f32, tag="vp", bufs=1, name="vp")
                    for kc in range(KC):
                        if mc == 0 and kc == 0:
                            xt = xt00
                        else:
                            xt = xt_pool.tile([P, MCH], f16, tag="xt", name="xt")
                            P1ENG[kc % 2].dma_start(xt, xT_d[bass.ts(kc, P), msl])
                        st, sp = kc == 0, kc == KC - 1
                        nc.tensor.matmul(qp, wq_sb[:, kc, :], xt, start=st, stop=sp)
                        nc.tensor.matmul(kp, wk_sb[:, kc, :], xt, start=st, stop=sp)
                        nc.tensor.matmul(vp, wv_sb[:, kc, :], xt, start=st, stop=sp)
                    # Q first: next group's first matmul reuses the q bank.
                    # Identity shares Exp's ACT table set (exp_and_others),
                    # so no mid-kernel table reload.
                    nc.scalar.add(QT[:, msl], qp, bq_sb)
                    nc.scalar.add(KT[:, msl], kp, bk_sb)
                    vt = vt_pool.tile([P, MCH], f16, name="vt")
                    nc.vector.tensor_scalar_add(vt, vp, bv_sb)
                    # previous group's V transposes fill the PE while this
                    # group's Q bias-add drains (next group's bank gate)
                    if pending_vt:
                        emit_vtrans(psum_p1, *pending_vt.pop(0))
                    pending_vt.append((mc, vt))
                while pending_vt:
                    emit_vtrans(psum_p1, *pending_vt.pop(0))

            # ---------------------------------------------------------------
            # phase 2: attention passes + deferred batch-1 projections +
            # out-projection, all interleaved
            # ---------------------------------------------------------------
            with (
                tc.tile_pool(name="psum_av", bufs=2, space="PSUM") as psum_av,
                tc.tile_pool(name="psum_op", bufs=2, space="PSUM") as psum_op,
            ):
                # --- normalization (no DRAM bounce) ---
                def emit_norm_w(b, pr, h, av_sb, q_off, width):
                    # av_sb [65, width]: rows 0:64 = unnormalized Y^T,
                    # row 64 = softmax denominator
                    qsl = bass.ds(b * S + pr * QW + q_off, width)
                    dnm = r_pool.tile([1, width], f32, tag="dnm", bufs=2, name="dnm")
                    nc.vector.tensor_copy(dnm, av_sb[D : D + 1, :])
                    rcp = r_pool.tile([1, width], f32, tag="rcp", bufs=2, name="rcp")
                    nc.vector.reciprocal_approx_fast(rcp, dnm)
                    rbs = r_pool.tile([D, width], f32, tag="rbs", bufs=2, name="rbs")
                    nc.gpsimd.partition_broadcast(rbs, rcp)
                    nc.vector.tensor_mul(
                        YT[D * h : D * (h + 1), qsl], av_sb[0:D, :], rbs
                    )

                def emit_norm(b, pr, h, av_sb):
                    emit_norm_w(b, pr, h, av_sb, 0, QW)

                # --- out-projection: one token-tile (128 tokens), K=128 ---
                def emit_outproj_tile(j, tail=False):
                    m0 = j * P
                    osb = out_pool.tile([P, E], f16, name="osb")
                    for ec in range(E // QCH):
                        esl = bass.ts(ec, QCH)
                        op = psum_op.tile([P, QCH], f32, tag="op", name="op")
                        nc.tensor.matmul(
                            op, YT[:, bass.ds(m0, P)], wo_sb[:, esl],
                            start=True, stop=True,
                        )
                        if tail and ec % 2 == 0:
                            nc.scalar.copy(osb[:, esl], op)  # ACT free at tail
                        else:
                            nc.vector.tensor_copy(osb[:, esl], op)
                    eng = nc.scalar if (tail and j % 2 == 1) else nc.sync
                    eng.dma_start(out_d[bass.ds(m0, P), :], osb)

                # --- deferred batch-1 projection filler units ---
                def make_proj_units():
                    units = []
                    shared = {}
                    for mc in range(S // MCH, M // MCH):
                        msl = bass.ts(mc, MCH)
                        state = {}
                        shared[mc] = state

                        def u_qk(mc=mc, msl=msl, state=state):
                            xts = []
                            for kc in range(KC):
                                xt = xt_pool.tile(
                                    [P, MCH], f16, tag="xt2", bufs=20, name="xt2"
                                )
                                nc.gpsimd.dma_start(xt, xT_d[bass.ts(kc, P), msl])
                                xts.append(xt)
                            state["xts"] = xts
                            qp = psum_op.tile([P, MCH], f32, tag="op", name="qp2")
                            for kc in range(KC):
                                nc.tensor.matmul(
                                    qp, wq_sb[:, kc, :], xts[kc],
                                    start=(kc == 0), stop=(kc == KC - 1),
                                )
                            nc.vector.tensor_scalar_add(QT[:, msl], qp, bq_sb)

                        def u_k(mc=mc, msl=msl, state=state):
                            kp = psum_op.tile([P, MCH], f32, tag="op", name="kp2")
                            for kc in range(KC):
                                nc.tensor.matmul(
                                    kp, wk_sb[:, kc, :], state["xts"][kc],
                                    start=(kc == 0), stop=(kc == KC - 1),
                                )
                            nc.vector.tensor_scalar_add(KT[:, msl], kp, bk_sb)

                        def u_v(mc=mc, msl=msl, state=state):
                            vp = psum_op.tile([P, MCH], f32, tag="op", name="vp2")
                            for kc in range(KC):
                                nc.tensor.matmul(
                                    vp, wv_sb[:, kc, :], state["xts"][kc],
                                    start=(kc == 0), stop=(kc == KC - 1),
                                )
                            vt = vt_pool.tile([P, MCH], f16, name="vt2", tag="vt2")
                            nc.vector.tensor_scalar_add(vt, vp, bv_sb)
                            state["vt"] = vt

                        def u_t(mc=mc, state=state):
                            vt = state["vt"]
                            for j in range(MCH // P):
                                mt = mc * (MCH // P) + j
                                vtp = psum_op.tile([P, P], f16, tag="op", name="vtp2")
                                nc.tensor.transpose(vtp, vt[:, bass.ts(j, P)], ident)
                                nc.vector.tensor_copy(Vtm[:, mt, 0:D], vtp[:, 0:D])
                                nc.vector.tensor_copy(
                                    Vtm[:, mt, D + 1 : 2 * D + 1], vtp[:, D : 2 * D]
                                )

                        units.append((mc, u_qk, u_k, u_v, u_t))
                    # interleave so at most ~2 chunks' xts are alive and
                    # K/Q(first half) finish before pass 4 reads batch 1
                    c0, c1, c2, c3 = units
                    return [
                        c0[1], c0[2], c1[1], c1[2], c0[3], c0[4],
                        c2[1], c2[2], c1[3], c1[4],
                        c3[1], c3[2], c2[3], c2[4], c3[3], c3[4],
                    ]

                proj_filler = make_proj_units()
                filler = []
                pending_norm = []

                def pop_filler():
                    if proj_filler:
                        proj_filler.pop(0)()
                    elif filler:
                        filler.pop(0)()

                # h innermost: both heads of a (b, pr) region complete
                # back-to-back, releasing out-proj tiles as early as possible
                passes = [
                    (b, pr, h) for b in range(B) for pr in range(2) for h in range(2)
                ]
                for pi, (b, pr, h) in enumerate(passes):
                    if pi == 4:
                        # batch-1 QT/KT/Vtm reads start here: all deferred
                        # projections must be emitted (normally already
                        # drained by the t%3 pacing - this is a safety net)
                        while proj_filler:
                            proj_filler.pop(0)()
                    last = pi == len(passes) - 1
                    dsl = bass.ds(D * h, D)
                    vcs = bass.ds(h * (D + 1), D + 1)  # [V_h | ones] cols
                    q0 = b * S + pr * QW
                    avs = [
                        psum_av.tile([D + 1, QCH], f32, tag="av", name="av")
                        for _ in range(2)
                    ]

                    def sc_exp(t, b=b, dsl=dsl, q0=q0):
                        # score matmuls + exp for key tile t; ACT consumes
                        # the sc psum, PE runs two tiles ahead (ring of 2)
                        ksl = bass.ds(b * S + t * P, P)
                        lhs_k = KT[dsl, ksl]
                        sc = psum_sc.tile([P, QW], f32, tag="sc", name="sc")
                        for qi in range(2):
                            nc.tensor.matmul(
                                sc[:, bass.ts(qi, QCH)],
                                lhs_k,
                                QT[dsl, bass.ds(q0 + qi * QCH, QCH)],
                                start=True, stop=True,
                            )
                        pt = pt_pool.tile([P, QW], f16, tag="pt", name="pt")
                        nc.scalar.activation(
                            pt, sc, Exp,
                            bias=mask_sb[:, b * 16 + t : b * 16 + t + 1],
                            scale=1.0,
                        )
                        return pt

                    # two-tile prologue keeps exp(t+1) queued on ACT while
                    # av(t)/fillers run on the PE
                    pts = {0: sc_exp(0), 1: sc_exp(1)}
                    kept = []
                    for t in range(NKT):
                        pt = pts.pop(t)
                        lhs_v = Vtm[:, b * 16 + t, vcs]
                        st, sp = t == 0, t == NKT - 1
                        nc.tensor.matmul(
                            avs[0], lhs_v, pt[:, 0:QCH], start=st, stop=sp
                        )
                        if not last:
                            nc.tensor.matmul(
                                avs[1], lhs_v, pt[:, QCH:QW], start=st, stop=sp
                            )
                        else:
                            kept.append(pt)  # qi1 deferred to the tail loop
                        if t + 2 < NKT:
                            pts[t + 2] = sc_exp(t + 2)
                        # previous pass's normalization (DVE/GPSIMD only)
                        if t == 1 and pending_norm:
                            args = pending_norm.pop(0)
                            emit_norm(*args)
                            bn, prn, hn = args[0], args[1], args[2]
                            if hn == 1:
                                # region (bn, prn) fully normalized
                                filler.extend(
                                    (lambda jj=j: emit_outproj_tile(jj))
                                    for j in range(
                                        (bn * S + prn * QW) // P,
                                        (bn * S + (prn + 1) * QW) // P,
                                    )
                                )
                        if (t % 5 == 4) if last else (t % 3 == 2):
                            pop_filler()
                    if not last:
                        av_sb = r_pool.tile(
                            [D + 1, QW], f32, tag="avsb", bufs=2, name="avsb"
                        )
                        for qi in range(2):
                            nc.vector.tensor_copy(
                                av_sb[:, bass.ts(qi, QCH)], avs[qi]
                            )
                        pending_norm.append((b, pr, h, av_sb))

                # ---- tail: last pass (1,1,1) qi=1 AV + final norms/tiles ----
                b, pr, h = passes[-1]
                vcs = bass.ds(h * (D + 1), D + 1)
                j0 = (b * S + pr * QW) // P

                def emit_outproj_tail(j, osb4, slot, eng):
                    # tail tiles reuse the drained score-psum tag: one
                    # 2-bank tile, two MMs, a single [128,1024] cast copy
                    m0 = j * P
                    op2 = psum_sc.tile([P, QW], f32, tag="sc", name="sc_op")
                    for ec in range(E // QCH):
                        nc.tensor.matmul(
                            op2[:, bass.ts(ec, QCH)],
                            YT[:, bass.ds(m0, P)], wo_sb[:, bass.ts(ec, QCH)],
                            start=True, stop=True,
                        )
                    if eng == 0:
                        nc.vector.tensor_copy(osb4[:, slot, :], op2)
                    else:
                        nc.scalar.copy(osb4[:, slot, :], op2)

                def dma_tiles(jlo, osb4, n):
                    dst = out_d[bass.ds(jlo * P, n * P), :].rearrange(
                        "(i p) e -> p i e", p=P
                    )
                    nc.sync.dma_start(dst, osb4[:, 0:n, :])

                av_sb0 = r_pool.tile([D + 1, QCH], f32, tag="avs0", bufs=1, name="avs0")
                nc.vector.tensor_copy(av_sb0, avs[0])
                for t in range(NKT):
                    nc.tensor.matmul(
                        avs[1], Vtm[:, b * 16 + t, vcs], kept[t][:, QCH:QW],
                        start=(t == 0), stop=(t == NKT - 1),
                    )
                    if t == 0:
                        emit_norm_w(b, pr, h, av_sb0, 0, QCH)
                    if t == 2:
                        # qi0 region tiles become poppable fillers
                        osb4a = out_pool.tile(
                            [P, 4, E], f16, tag="tosb", bufs=2, name="osb4a"
                        )
                        filler.extend(
                            (lambda jj=j, s=s: emit_outproj_tail(
                                jj, osb4a, s, s % 2))
                            for s, j in enumerate(range(j0, j0 + 4))
                        )
                        filler.append(lambda: dma_tiles(j0, osb4a, 4))
                    if t >= 4 and t % 3 == 1:
                        pop_filler()
                # stage + normalize qi1, then the final four tiles
                av_sb1 = r_pool.tile([D + 1, QCH], f32, tag="avs0", bufs=1, name="avs1")
                nc.vector.tensor_copy(av_sb1, avs[1])
                pop_filler()
                emit_norm_w(b, pr, h, av_sb1, QCH, QCH)
                while proj_filler:
                    proj_filler.pop(0)()
                while filler:
                    filler.pop(0)()
                osb4b = out_pool.tile([P, 4, E], f16, tag="tosb", bufs=2, name="osb4b")
                for s, j in enumerate(range(j0 + 4, j0 + 8)):
                    emit_outproj_tail(j, osb4b, s, s % 2)
                dma_tiles(j0 + 4, osb4b, 4)

    nc.compile()
    return nc


def kernel(x, mask, Wq, bq, Wk, bk, Wv, bv, Wo, bo):
    global LAST_RESULTS, _PROGRAM
    from concourse.bass_utils import run_bass_kernel_spmd

    if _PROGRAM is None:
        _PROGRAM = _build_program()
    nc = _PROGRAM

    f16 = np.float16
    x = np.asarray(x, dtype=np.float32)
    mask = np.asarray(mask)
    f32c = lambda a: np.ascontiguousarray(np.asarray(a, dtype=np.float32))

    xT = np.ascontiguousarray(x.reshape(M, E).T.astype(f16))     # [E, M]
    maskf = np.where(mask, np.float32(NEG), np.float32(0.0)).astype(np.float32)
    maskT = np.ascontiguousarray(
        maskf.reshape(B, 16, P).transpose(2, 0, 1).reshape(P, B * 16)
    )
    scale = np.float32(1.0 / np.sqrt(D))

    in_maps = []
    for c in range(NCORES):
        csl = slice(P * c, P * (c + 1))
        wq_c = (np.asarray(Wq, dtype=np.float32)[:, csl] * scale).astype(f16)
        wk_c = np.asarray(Wk, dtype=np.float32)[:, csl].astype(f16)
        wv_c = np.asarray(Wv, dtype=np.float32)[:, csl].astype(f16)
        in_maps.append(
            {
                "xT": xT,
                "wq": np.ascontiguousarray(wq_c.reshape(KC, P, P).transpose(1, 0, 2)),
                "wk": np.ascontiguousarray(wk_c.reshape(KC, P, P).transpose(1, 0, 2)),
                "wv": np.ascontiguousarray(wv_c.reshape(KC, P, P).transpose(1, 0, 2)),
                "wo": np.ascontiguousarray(
                    np.asarray(Wo, dtype=np.float32)[csl, :].astype(f16)
                ),
                "bq": f32c(np.asarray(bq)[csl] * scale).reshape(P, 1),
                "bk": f32c(np.asarray(bk)[csl]).reshape(P, 1),
                "bv": f32c(np.asarray(bv)[csl]).reshape(P, 1),
                "maskT": maskT,
            }
        )

    trace = bool(os.environ.get("KERNEL_TRACE"))
    LAST_RESULTS = run_bass_kernel_spmd(
        nc, in_maps, list(range(NCORES)), trace=trace
    )

    acc = np.zeros((M, E), dtype=np.float64)
    for res in LAST_RESULTS.results:
        acc += res["out"].astype(np.float64)
    out = (acc + np.asarray(bo, dtype=np.float64)[None, :]).astype(np.float32)
    return out.reshape(B, S, E)
